# revision 23
# baseline (speedup 1.0000x reference)
"""Coverage-attention kernel for Trainium2, data-parallel over batch on 8 NeuronCores.

Reference computation (per batch b):
    cat   = [state_h; state_c]                      (2D,)
    cov   = (coverage + 1e-3) / sum(coverage + 1e-3)
    A     = tanh(context @ W_h + cat @ W_s + cov[:,None]*W_cov + biases)   (S, D)
    e     = A @ v_w                                  (S,)
    attn  = exp(e)*mask / sum(exp(e)*mask)           (softmax shift cancels)
    attn_h = attn @ context                          (D,)

Sharding: batch 32 -> 4 per core; weights replicated. Heavy math in bf16 on
TensorE; fp32 accumulation in PSUM.
"""

import os
import sys

import numpy as np

for _p in ("/opt/trn_rl_repo", "/root/.axon_site/_ro/trn_rl_repo"):
    if os.path.isdir(_p) and _p not in sys.path:
        sys.path.append(_p)

B, S, D = 32, 2048, 1024
NCORES = 8
BPC = B // NCORES          # batches per core
P = 128                    # partitions
KC = D // P                # 8 contraction chunks for the D x D matmul
NT = S // P                # 16 sequence tiles per batch
NH = 512                   # matmul moving free-dim (one PSUM bank of fp32)


def build_nc(bpc=BPC, nt=NT):
    """Build the per-core Bass graph. Identical on all cores (pure SPMD)."""
    import concourse.bass as bass  # noqa: F401
    import concourse.tile as tile
    from concourse import bacc, mybir
    from concourse.masks import make_identity
    from contextlib import ExitStack

    fp32 = mybir.dt.float32
    bf16 = mybir.dt.bfloat16
    i32 = mybir.dt.int32
    s = nt * P
    Tanh = mybir.ActivationFunctionType.Tanh
    Exp = mybir.ActivationFunctionType.Exp
    mult = mybir.AluOpType.mult
    add = mybir.AluOpType.add

    nc = bacc.Bacc(None, target_bir_lowering=False, debug=False)

    ctx_in = nc.declare_dram_parameter("context", [bpc, s, D], fp32, isOutput=False)
    sh_in = nc.declare_dram_parameter("state_h", [bpc, D], fp32, isOutput=False)
    sc_in = nc.declare_dram_parameter("state_c", [bpc, D], fp32, isOutput=False)
    mask_in = nc.declare_dram_parameter("context_mask", [bpc, s], i32, isOutput=False)
    cov_in = nc.declare_dram_parameter("coverage", [bpc, s], fp32, isOutput=False)
    wh_in = nc.declare_dram_parameter("W_h", [D, D], fp32, isOutput=False)
    bh_in = nc.declare_dram_parameter("b_h", [1, D], fp32, isOutput=False)
    ws_in = nc.declare_dram_parameter("W_s", [2 * D, D], fp32, isOutput=False)
    bs_in = nc.declare_dram_parameter("b_s", [1, D], fp32, isOutput=False)
    wcov_in = nc.declare_dram_parameter("W_cov", [1, D], fp32, isOutput=False)
    bcov_in = nc.declare_dram_parameter("b_cov", [1, D], fp32, isOutput=False)
    vw_in = nc.declare_dram_parameter("v_w", [1, D], fp32, isOutput=False)

    ah_out = nc.declare_dram_parameter("attn_h", [bpc, D], fp32, isOutput=True)
    attn_out = nc.declare_dram_parameter("attn", [bpc, s], fp32, isOutput=True)

    # DRAM views with the sequence dim split (t p): p fastest -> contiguous
    mask_v = mask_in[:].rearrange("b (t p) -> b t p", p=P)
    attn_v = attn_out[:].rearrange("b (t p) -> b t p", p=P)
    wh_v = wh_in[:].rearrange("(k p) n -> p k n", p=P)

    with tile.TileContext(nc) as tc, ExitStack() as ctx:
        consts = ctx.enter_context(tc.tile_pool(name="consts", bufs=1))
        ctxp = ctx.enter_context(tc.tile_pool(name="ctxnat", bufs=2 * nt))
        trp = ctx.enter_context(tc.tile_pool(name="ctxT", bufs=3))
        dramp = ctx.enter_context(tc.tile_pool(name="dram", bufs=2, space="DRAM"))
        actp = ctx.enter_context(tc.tile_pool(name="acts", bufs=2))
        scr = ctx.enter_context(tc.tile_pool(name="scratch", bufs=2))
        bpool = ctx.enter_context(tc.tile_pool(name="perbatch", bufs=2))
        outp = ctx.enter_context(tc.tile_pool(name="outs", bufs=2))
        ps_main = ctx.enter_context(tc.tile_pool(name="psmain", bufs=2, space="PSUM"))
        ps_tr = ctx.enter_context(tc.tile_pool(name="pstr", bufs=2, space="PSUM"))
        ps_attn = ctx.enter_context(tc.tile_pool(name="psattn", bufs=1, space="PSUM"))

        # ---- constants -------------------------------------------------
        ident = consts.tile([P, P], bf16, tag="ident")
        make_identity(nc, ident)

        whb = consts.tile([P, KC, D], bf16, tag="whb")
        nc.gpsimd.dma_start(out=whb, in_=wh_v)

        vw_b = consts.tile([P, D], bf16, tag="vwb")
        nc.gpsimd.dma_start(out=vw_b, in_=vw_in[:].to_broadcast([P, D]))

        ones_col = consts.tile([P, 1], bf16, tag="ones")
        nc.vector.memset(ones_col, 1.0)
        ones_row = consts.tile([1, P], fp32, tag="onesrow")
        nc.vector.memset(ones_row, 1.0)

        # persistent results of the prologue, consumed per batch
        wstate_b = consts.tile([bpc, D], bf16, tag="wstateb")
        cov_nb = consts.tile([bpc, s], bf16, tag="covnb")

        # prologue-only tiles live in a pool that closes before the main
        # loop so their SBUF space is recycled
        with tc.tile_pool(name="prologue", bufs=1) as pro, \
             tc.tile_pool(name="wstream", bufs=2) as wspool:
            # w_state = cat(state_h, state_c) @ W_s + (b_s + b_h + b_cov)
            cat_f = pro.tile([bpc, 2 * D], fp32, tag="catf")
            nc.sync.dma_start(out=cat_f[:, 0:D], in_=sh_in[:])
            nc.sync.dma_start(out=cat_f[:, D : 2 * D], in_=sc_in[:])
            cat_b = pro.tile([bpc, 2 * D], bf16, tag="catb")
            nc.vector.tensor_copy(cat_b, cat_f)

            cstT = pro.tile([P, 2 * KC, bpc], bf16, tag="cstT")
            for k in range(2 * KC):
                pt = ps_tr.tile([P, bpc], bf16, tag="tr")
                nc.tensor.transpose(pt, cat_b[:, k * P : (k + 1) * P], ident[:bpc, :bpc])
                nc.vector.tensor_copy(cstT[:, k, :], pt)

            psw = ps_main.tile([bpc, D], fp32, tag="main")
            for k in range(2 * KC):
                wst = wspool.tile([P, D], bf16, tag="ws")
                nc.gpsimd.dma_start(out=wst, in_=ws_in[k * P : (k + 1) * P, :])
                for n in range(D // NH):
                    nc.tensor.matmul(
                        psw[:, n * NH : (n + 1) * NH],
                        lhsT=cstT[:, k, :],
                        rhs=wst[:, n * NH : (n + 1) * NH],
                        start=(k == 0),
                        stop=(k == 2 * KC - 1),
                    )
            bias_t = pro.tile([bpc, D], fp32, tag="biast")
            nc.gpsimd.dma_start(out=bias_t, in_=bs_in[:].to_broadcast([bpc, D]))
            tmp_b = pro.tile([bpc, D], fp32, tag="biastmp")
            nc.gpsimd.dma_start(out=tmp_b, in_=bh_in[:].to_broadcast([bpc, D]))
            nc.vector.tensor_add(bias_t, bias_t, tmp_b)
            tmp_b2 = pro.tile([bpc, D], fp32, tag="biastmp")
            nc.gpsimd.dma_start(out=tmp_b2, in_=bcov_in[:].to_broadcast([bpc, D]))
            nc.vector.tensor_add(bias_t, bias_t, tmp_b2)
            wstate = pro.tile([bpc, D], fp32, tag="wstate")
            nc.vector.tensor_add(wstate, psw, bias_t)
            nc.vector.tensor_copy(wstate_b, wstate)

            # coverage normalization
            cov_f = pro.tile([bpc, s], fp32, tag="covf")
            nc.sync.dma_start(out=cov_f, in_=cov_in[:])
            cov_sum = pro.tile([bpc, 1], fp32, tag="covsum")
            nc.vector.tensor_reduce(out=cov_sum, in_=cov_f, axis=mybir.AxisListType.X, op=add)
            nc.vector.tensor_scalar_add(cov_sum, cov_sum, 0.001 * s)
            cov_rcp = pro.tile([bpc, 1], fp32, tag="covrcp")
            nc.vector.reciprocal(cov_rcp, cov_sum)
            nc.vector.tensor_scalar(
                out=cov_nb, in0=cov_f, scalar1=0.001, scalar2=cov_rcp, op0=add, op1=mult
            )

        # ---- main loop --------------------------------------------------
        for b in range(bpc):
            # rank-1 bias operands for this batch
            r1b = bpool.tile([2, D], bf16, tag="r1b")
            nc.gpsimd.dma_start(out=r1b[0:1, :], in_=wstate_b[b : b + 1, :])
            nc.gpsimd.dma_start(out=r1b[1:2, :], in_=wcov_in[:])
            lcovb = bpool.tile([2, s], bf16, tag="lcovb")
            nc.vector.memset(lcovb[0:1, :], 1.0)
            nc.scalar.dma_start(out=lcovb[1:2, :], in_=cov_nb[b : b + 1, :])
            # mask -> (P, nt) bf16 for this batch
            mi = bpool.tile([nt, P], i32, tag="maski")
            nc.scalar.dma_start(out=mi, in_=mask_v[b])
            mf = bpool.tile([nt, P], fp32, tag="maskf")
            nc.vector.tensor_copy(mf, mi)
            mb = bpool.tile([nt, P], bf16, tag="maskb")
            nc.vector.tensor_copy(mb, mf)
            pmt = ps_tr.tile([P, nt], bf16, tag="tr")
            nc.tensor.transpose(pmt, mb, ident[:nt, :nt])
            mask_t = bpool.tile([P, nt], bf16, tag="maskt")
            nc.vector.tensor_copy(mask_t, pmt)

            # phase A/B: load context bf16, mirror to DRAM scratch, and read
            # back transposed via the DMA xbar. Half-batch granularity so the
            # first transposes start after 8 tiles instead of 16.
            scratch = dramp.tile([s, D], bf16, tag="scratch")
            nat_tiles = []
            ctxT_h = []
            hh = nt // 2 * P
            for h in range(2):
                for t in range(nt // 2 * h, nt // 2 * (h + 1)):
                    cn = ctxp.tile([P, D], bf16, tag="nat")
                    nat_tiles.append(cn)
                    nc.gpsimd.dma_start(out=cn, in_=ctx_in[b, t * P : (t + 1) * P, :])
                    nc.gpsimd.dma_start(out=scratch[t * P : (t + 1) * P, :], in_=cn)
                cth = trp.tile([P, KC, hh], bf16, tag="ctxTh")
                ctxT_h.append(cth)
                for k in range(KC):
                    nc.sync.dma_start(
                        out=cth[:, k, :],
                        in_=scratch[h * hh : (h + 1) * hh, k * P : (k + 1) * P],
                        transpose=True,
                    )
            # phase C: per-tile matmuls + fused softmax-numerator + attn_h accum.
            # No max-subtraction is needed (|e| is small and the shift cancels),
            # so p = exp(e)*mask accumulates into attn_h immediately and the
            # context tile is released after one use.
            p_buf = bpool.tile([P, nt], bf16, tag="pbuf")
            pa = ps_attn.tile([1, D], fp32, tag="attn")
            for t in range(nt):
                pm = ps_main.tile([P, D], fp32, tag="main")
                for n in range(D // NH):
                    nc.tensor.matmul(
                        pm[:, n * NH : (n + 1) * NH],
                        lhsT=lcovb[:, t * P : (t + 1) * P],
                        rhs=r1b[:, n * NH : (n + 1) * NH],
                        start=True,
                        stop=False,
                    )
                for k in range(KC):
                    for n in range(D // NH):
                        nc.tensor.matmul(
                            pm[:, n * NH : (n + 1) * NH],
                            lhsT=ctxT_h[t // (nt // 2)][:, k, (t % (nt // 2)) * P : (t % (nt // 2) + 1) * P],
                            rhs=whb[:, k, n * NH : (n + 1) * NH],
                            start=False,
                            stop=(k == KC - 1),
                        )

                act = actp.tile([P, D], bf16, tag="act")
                nc.scalar.activation(act, pm, Tanh)
                ttro = scr.tile([P, D], bf16, tag="ttro")
                nc.vector.tensor_mul(ttro, act, vw_b)
                e_col = scr.tile([P, 1], fp32, tag="ecol")
                nc.vector.tensor_reduce(
                    out=e_col, in_=ttro, axis=mybir.AxisListType.X, op=add
                )
                nc.scalar.activation(p_buf[:, t : t + 1], e_col, Exp)
                nc.vector.tensor_mul(
                    p_buf[:, t : t + 1], p_buf[:, t : t + 1], mask_t[:, t : t + 1]
                )
                for tp in ([t - 1] if t >= 1 else []) + ([t] if t == nt - 1 else []):
                    for n in range(D // NH):
                        nc.tensor.matmul(
                            pa[:, n * NH : (n + 1) * NH],
                            lhsT=p_buf[:, tp : tp + 1],
                            rhs=nat_tiles[tp][:, n * NH : (n + 1) * NH],
                            start=(tp == 0),
                            stop=(tp == nt - 1),
                        )

            # Z = sum over all positions (partition-sum via ones matmul)
            zp = ps_tr.tile([1, nt], fp32, tag="tr")
            nc.tensor.matmul(zp, lhsT=ones_col, rhs=p_buf, start=True, stop=True)
            zs = outp.tile([1, 1], fp32, tag="zs")
            nc.vector.tensor_reduce(out=zs, in_=zp, axis=mybir.AxisListType.X, op=add)
            # broadcast Z to all partitions via a K=1 ones matmul, then 1/Z
            zbp = ps_tr.tile([P, 1], fp32, tag="tr")
            nc.tensor.matmul(zbp, lhsT=ones_row, rhs=zs, start=True, stop=True)
            rzb = outp.tile([P, 1], fp32, tag="rzb")
            nc.vector.reciprocal(rzb, zbp)
            ah = outp.tile([1, D], fp32, tag="ah")
            nc.vector.tensor_scalar_mul(ah, pa, rzb[0:1, :])
            nc.scalar.dma_start(out=ah_out[b : b + 1, :], in_=ah)

            # attn row out: scale (bf16), transpose to (t, p) so DRAM writes are
            # contiguous, cast to fp32 on the PSUM->SBUF copy
            arow = outp.tile([P, nt], bf16, tag="arow")
            nc.vector.tensor_scalar_mul(arow, p_buf, rzb)
            pat = ps_tr.tile([nt, P], bf16, tag="tr")
            nc.tensor.transpose(pat, arow, ident)
            arow_t = outp.tile([nt, P], fp32, tag="arowt")
            nc.vector.tensor_copy(arow_t, pat)
            nc.scalar.dma_start(out=attn_v[b], in_=arow_t)

    nc.compile()
    return nc


_NC_CACHE = {}


def _get_nc():
    if "nc" not in _NC_CACHE:
        _NC_CACHE["nc"] = build_nc()
    return _NC_CACHE["nc"]


def make_in_maps(inputs, bpc=BPC, ncores=NCORES):
    f = lambda x: np.ascontiguousarray(np.asarray(x, dtype=np.float32))
    shared = {
        "W_h": f(inputs["W_h"]),
        "b_h": f(inputs["b_h"]).reshape(1, D),
        "W_s": f(inputs["W_s"]),
        "b_s": f(inputs["b_s"]).reshape(1, D),
        "W_cov": f(inputs["W_cov"]).reshape(1, D),
        "b_cov": f(inputs["b_cov"]).reshape(1, D),
        "v_w": f(inputs["v_w"]).reshape(1, D),
    }
    mask = np.ascontiguousarray(np.asarray(inputs["context_mask"], dtype=np.int32))
    in_maps = []
    for c in range(ncores):
        sl = slice(c * bpc, (c + 1) * bpc)
        m = dict(shared)
        m["context"] = f(inputs["context"][sl])
        m["state_h"] = f(inputs["state_h"][sl])
        m["state_c"] = f(inputs["state_c"][sl])
        m["coverage"] = f(inputs["coverage"][sl])
        m["context_mask"] = mask[sl]
        in_maps.append(m)
    return in_maps


def gather_outputs(results):
    attn_h = np.concatenate([np.asarray(r["attn_h"]) for r in results], axis=0)
    attn = np.concatenate([np.asarray(r["attn"]) for r in results], axis=0)
    return attn_h.astype(np.float32), attn.astype(np.float32)


def kernel(**inputs):
    from concourse.bass_utils import run_bass_kernel_spmd

    nc = _get_nc()
    in_maps = make_in_maps(inputs)
    res = run_bass_kernel_spmd(nc, in_maps, core_ids=list(range(NCORES)))
    return gather_outputs(res.results)


# revision 24
# speedup vs baseline: 1.2293x; 1.2293x over previous
"""Coverage-attention kernel for Trainium2, data-parallel over batch on 8 NeuronCores.

Reference computation (per batch b):
    cat   = [state_h; state_c]                      (2D,)
    cov   = (coverage + 1e-3) / sum(coverage + 1e-3)
    A     = tanh(context @ W_h + cat @ W_s + cov[:,None]*W_cov + biases)   (S, D)
    e     = A @ v_w                                  (S,)
    attn  = exp(e)*mask / sum(exp(e)*mask)           (softmax shift cancels)
    attn_h = attn @ context                          (D,)

Sharding: batch 32 -> 4 per core; weights replicated. Heavy math in bf16 on
TensorE; fp32 accumulation in PSUM.
"""

import os
import sys

import numpy as np

for _p in ("/opt/trn_rl_repo", "/root/.axon_site/_ro/trn_rl_repo"):
    if os.path.isdir(_p) and _p not in sys.path:
        sys.path.append(_p)

B, S, D = 32, 2048, 1024
NCORES = 8
BPC = B // NCORES          # batches per core
P = 128                    # partitions
KC = D // P                # 8 contraction chunks for the D x D matmul
NT = S // P                # 16 sequence tiles per batch
NH = 512                   # matmul moving free-dim (one PSUM bank of fp32)


def build_nc(bpc=BPC, nt=NT):
    """Build the per-core Bass graph. Identical on all cores (pure SPMD)."""
    import concourse.bass as bass  # noqa: F401
    import concourse.tile as tile
    from concourse import bacc, mybir
    from concourse.masks import make_identity
    from contextlib import ExitStack

    fp32 = mybir.dt.float32
    bf16 = mybir.dt.bfloat16
    i32 = mybir.dt.int32
    s = nt * P
    Tanh = mybir.ActivationFunctionType.Tanh
    Exp = mybir.ActivationFunctionType.Exp
    mult = mybir.AluOpType.mult
    add = mybir.AluOpType.add

    nc = bacc.Bacc(None, target_bir_lowering=False, debug=False)

    ctx_in = nc.declare_dram_parameter("context", [bpc, s, D], fp32, isOutput=False)
    sh_in = nc.declare_dram_parameter("state_h", [bpc, D], fp32, isOutput=False)
    sc_in = nc.declare_dram_parameter("state_c", [bpc, D], fp32, isOutput=False)
    mask_in = nc.declare_dram_parameter("context_mask", [bpc, s], i32, isOutput=False)
    cov_in = nc.declare_dram_parameter("coverage", [bpc, s], fp32, isOutput=False)
    wh_in = nc.declare_dram_parameter("W_h", [D, D], fp32, isOutput=False)
    bh_in = nc.declare_dram_parameter("b_h", [1, D], fp32, isOutput=False)
    ws_in = nc.declare_dram_parameter("W_s", [2 * D, D], fp32, isOutput=False)
    bs_in = nc.declare_dram_parameter("b_s", [1, D], fp32, isOutput=False)
    wcov_in = nc.declare_dram_parameter("W_cov", [1, D], fp32, isOutput=False)
    bcov_in = nc.declare_dram_parameter("b_cov", [1, D], fp32, isOutput=False)
    vw_in = nc.declare_dram_parameter("v_w", [1, D], fp32, isOutput=False)

    ah_out = nc.declare_dram_parameter("attn_h", [bpc, D], fp32, isOutput=True)
    attn_out = nc.declare_dram_parameter("attn", [bpc, s], fp32, isOutput=True)

    # DRAM views with the sequence dim split (t p): p fastest -> contiguous
    mask_v = mask_in[:].rearrange("b (t p) -> b t p", p=P)
    attn_v = attn_out[:].rearrange("b (t p) -> b t p", p=P)
    wh_v = wh_in[:].rearrange("(k p) n -> p k n", p=P)

    with tile.TileContext(nc) as tc, ExitStack() as ctx:
        consts = ctx.enter_context(tc.tile_pool(name="consts", bufs=1))
        ctxp = ctx.enter_context(tc.tile_pool(name="ctxnat", bufs=2 * nt))
        trp = ctx.enter_context(tc.tile_pool(name="ctxT", bufs=3))
        dramp = ctx.enter_context(tc.tile_pool(name="dram", bufs=2, space="DRAM"))
        actp = ctx.enter_context(tc.tile_pool(name="acts", bufs=2))
        scr = ctx.enter_context(tc.tile_pool(name="scratch", bufs=2))
        bpool = ctx.enter_context(tc.tile_pool(name="perbatch", bufs=2))
        outp = ctx.enter_context(tc.tile_pool(name="outs", bufs=2))
        ps_main = ctx.enter_context(tc.tile_pool(name="psmain", bufs=2, space="PSUM"))
        ps_tr = ctx.enter_context(tc.tile_pool(name="pstr", bufs=2, space="PSUM"))
        ps_attn = ctx.enter_context(tc.tile_pool(name="psattn", bufs=1, space="PSUM"))

        # ---- constants -------------------------------------------------
        ident = consts.tile([P, P], bf16, tag="ident")
        make_identity(nc, ident)

        whb = consts.tile([P, KC, D], bf16, tag="whb")
        nc.gpsimd.dma_start(out=whb, in_=wh_v)

        vw_b = consts.tile([P, D], bf16, tag="vwb")
        nc.gpsimd.dma_start(out=vw_b, in_=vw_in[:].to_broadcast([P, D]))

        ones_col = consts.tile([P, 1], bf16, tag="ones")
        nc.vector.memset(ones_col, 1.0)
        ones_row = consts.tile([1, P], fp32, tag="onesrow")
        nc.vector.memset(ones_row, 1.0)

        # persistent results of the prologue, consumed per batch
        wstate_b = consts.tile([bpc, D], bf16, tag="wstateb")
        cov_nb = consts.tile([bpc, s], bf16, tag="covnb")

        # prologue-only tiles live in a pool that closes before the main
        # loop so their SBUF space is recycled
        with tc.tile_pool(name="prologue", bufs=1) as pro, \
             tc.tile_pool(name="wstream", bufs=2) as wspool:
            # w_state = cat(state_h, state_c) @ W_s + (b_s + b_h + b_cov)
            cat_f = pro.tile([bpc, 2 * D], fp32, tag="catf")
            nc.sync.dma_start(out=cat_f[:, 0:D], in_=sh_in[:])
            nc.sync.dma_start(out=cat_f[:, D : 2 * D], in_=sc_in[:])
            cat_b = pro.tile([bpc, 2 * D], bf16, tag="catb")
            nc.vector.tensor_copy(cat_b, cat_f)

            cstT = pro.tile([P, 2 * KC, bpc], bf16, tag="cstT")
            for k in range(2 * KC):
                pt = ps_tr.tile([P, bpc], bf16, tag="tr")
                nc.tensor.transpose(pt, cat_b[:, k * P : (k + 1) * P], ident[:bpc, :bpc])
                nc.vector.tensor_copy(cstT[:, k, :], pt)

            psw = ps_main.tile([bpc, D], fp32, tag="main")
            for k in range(2 * KC):
                wst = wspool.tile([P, D], bf16, tag="ws")
                nc.gpsimd.dma_start(out=wst, in_=ws_in[k * P : (k + 1) * P, :])
                for n in range(D // NH):
                    nc.tensor.matmul(
                        psw[:, n * NH : (n + 1) * NH],
                        lhsT=cstT[:, k, :],
                        rhs=wst[:, n * NH : (n + 1) * NH],
                        start=(k == 0),
                        stop=(k == 2 * KC - 1),
                    )
            bias_t = pro.tile([bpc, D], fp32, tag="biast")
            nc.gpsimd.dma_start(out=bias_t, in_=bs_in[:].to_broadcast([bpc, D]))
            tmp_b = pro.tile([bpc, D], fp32, tag="biastmp")
            nc.gpsimd.dma_start(out=tmp_b, in_=bh_in[:].to_broadcast([bpc, D]))
            nc.vector.tensor_add(bias_t, bias_t, tmp_b)
            tmp_b2 = pro.tile([bpc, D], fp32, tag="biastmp")
            nc.gpsimd.dma_start(out=tmp_b2, in_=bcov_in[:].to_broadcast([bpc, D]))
            nc.vector.tensor_add(bias_t, bias_t, tmp_b2)
            wstate = pro.tile([bpc, D], fp32, tag="wstate")
            nc.vector.tensor_add(wstate, psw, bias_t)
            nc.vector.tensor_copy(wstate_b, wstate)

            # coverage normalization
            cov_f = pro.tile([bpc, s], fp32, tag="covf")
            nc.sync.dma_start(out=cov_f, in_=cov_in[:])
            cov_sum = pro.tile([bpc, 1], fp32, tag="covsum")
            nc.vector.tensor_reduce(out=cov_sum, in_=cov_f, axis=mybir.AxisListType.X, op=add)
            nc.vector.tensor_scalar_add(cov_sum, cov_sum, 0.001 * s)
            cov_rcp = pro.tile([bpc, 1], fp32, tag="covrcp")
            nc.vector.reciprocal(cov_rcp, cov_sum)
            nc.vector.tensor_scalar(
                out=cov_nb, in0=cov_f, scalar1=0.001, scalar2=cov_rcp, op0=add, op1=mult
            )

        # ---- main loop --------------------------------------------------
        for b in range(bpc):
            # rank-1 bias operands for this batch
            r1b = bpool.tile([2, D], bf16, tag="r1b")
            nc.gpsimd.dma_start(out=r1b[0:1, :], in_=wstate_b[b : b + 1, :])
            nc.gpsimd.dma_start(out=r1b[1:2, :], in_=wcov_in[:])
            lcovb = bpool.tile([2, s], bf16, tag="lcovb")
            nc.vector.memset(lcovb[0:1, :], 1.0)
            nc.scalar.dma_start(out=lcovb[1:2, :], in_=cov_nb[b : b + 1, :])
            # mask -> (P, nt) bf16 for this batch
            mi = bpool.tile([nt, P], i32, tag="maski")
            nc.scalar.dma_start(out=mi, in_=mask_v[b])
            mf = bpool.tile([nt, P], fp32, tag="maskf")
            nc.vector.tensor_copy(mf, mi)
            mb = bpool.tile([nt, P], bf16, tag="maskb")
            nc.vector.tensor_copy(mb, mf)
            pmt = ps_tr.tile([P, nt], bf16, tag="tr")
            nc.tensor.transpose(pmt, mb, ident[:nt, :nt])
            mask_t = bpool.tile([P, nt], bf16, tag="maskt")
            nc.vector.tensor_copy(mask_t, pmt)

            # phase A/B: load context bf16, mirror to DRAM scratch, and read
            # back transposed via the DMA xbar. Half-batch granularity so the
            # first transposes start after 8 tiles instead of 16.
            scratch = dramp.tile([s, D], bf16, tag="scratch")
            nat_tiles = []
            ctxT_h = []
            hh = nt // 2 * P
            for h in range(2):
                for t in range(nt // 2 * h, nt // 2 * (h + 1)):
                    cn = ctxp.tile([P, D], bf16, tag="nat")
                    nat_tiles.append(cn)
                    nc.gpsimd.dma_start(out=cn, in_=ctx_in[b, t * P : (t + 1) * P, :])
                    nc.scalar.dma_start(out=scratch[t * P : (t + 1) * P, :], in_=cn)
                cth = trp.tile([P, KC, hh], bf16, tag="ctxTh")
                ctxT_h.append(cth)
                for k in range(KC):
                    nc.sync.dma_start(
                        out=cth[:, k, :],
                        in_=scratch[h * hh : (h + 1) * hh, k * P : (k + 1) * P],
                        transpose=True,
                    )
            # phase C: per-tile matmuls + fused softmax-numerator + attn_h accum.
            # No max-subtraction is needed (|e| is small and the shift cancels),
            # so p = exp(e)*mask accumulates into attn_h immediately and the
            # context tile is released after one use.
            p_buf = bpool.tile([P, nt], bf16, tag="pbuf")
            pa = ps_attn.tile([1, D], fp32, tag="attn")
            for t in range(nt):
                pm = ps_main.tile([P, D], fp32, tag="main")
                for n in range(D // NH):
                    nc.tensor.matmul(
                        pm[:, n * NH : (n + 1) * NH],
                        lhsT=lcovb[:, t * P : (t + 1) * P],
                        rhs=r1b[:, n * NH : (n + 1) * NH],
                        start=True,
                        stop=False,
                    )
                for k in range(KC):
                    for n in range(D // NH):
                        nc.tensor.matmul(
                            pm[:, n * NH : (n + 1) * NH],
                            lhsT=ctxT_h[t // (nt // 2)][:, k, (t % (nt // 2)) * P : (t % (nt // 2) + 1) * P],
                            rhs=whb[:, k, n * NH : (n + 1) * NH],
                            start=False,
                            stop=(k == KC - 1),
                        )

                act = actp.tile([P, D], bf16, tag="act")
                nc.scalar.activation(act, pm, Tanh)
                ttro = scr.tile([P, D], bf16, tag="ttro")
                nc.vector.tensor_mul(ttro, act, vw_b)
                e_col = scr.tile([P, 1], fp32, tag="ecol")
                nc.vector.tensor_reduce(
                    out=e_col, in_=ttro, axis=mybir.AxisListType.X, op=add
                )
                nc.scalar.activation(p_buf[:, t : t + 1], e_col, Exp)
                nc.vector.tensor_mul(
                    p_buf[:, t : t + 1], p_buf[:, t : t + 1], mask_t[:, t : t + 1]
                )
                for tp in ([t - 1] if t >= 1 else []) + ([t] if t == nt - 1 else []):
                    for n in range(D // NH):
                        nc.tensor.matmul(
                            pa[:, n * NH : (n + 1) * NH],
                            lhsT=p_buf[:, tp : tp + 1],
                            rhs=nat_tiles[tp][:, n * NH : (n + 1) * NH],
                            start=(tp == 0),
                            stop=(tp == nt - 1),
                        )

            # Z = sum over all positions (partition-sum via ones matmul)
            zp = ps_tr.tile([1, nt], fp32, tag="tr")
            nc.tensor.matmul(zp, lhsT=ones_col, rhs=p_buf, start=True, stop=True)
            zs = outp.tile([1, 1], fp32, tag="zs")
            nc.vector.tensor_reduce(out=zs, in_=zp, axis=mybir.AxisListType.X, op=add)
            # broadcast Z to all partitions via a K=1 ones matmul, then 1/Z
            zbp = ps_tr.tile([P, 1], fp32, tag="tr")
            nc.tensor.matmul(zbp, lhsT=ones_row, rhs=zs, start=True, stop=True)
            rzb = outp.tile([P, 1], fp32, tag="rzb")
            nc.vector.reciprocal(rzb, zbp)
            ah = outp.tile([1, D], fp32, tag="ah")
            nc.vector.tensor_scalar_mul(ah, pa, rzb[0:1, :])
            nc.scalar.dma_start(out=ah_out[b : b + 1, :], in_=ah)

            # attn row out: scale (bf16), transpose to (t, p) so DRAM writes are
            # contiguous, cast to fp32 on the PSUM->SBUF copy
            arow = outp.tile([P, nt], bf16, tag="arow")
            nc.vector.tensor_scalar_mul(arow, p_buf, rzb)
            pat = ps_tr.tile([nt, P], bf16, tag="tr")
            nc.tensor.transpose(pat, arow, ident)
            arow_t = outp.tile([nt, P], fp32, tag="arowt")
            nc.vector.tensor_copy(arow_t, pat)
            nc.scalar.dma_start(out=attn_v[b], in_=arow_t)

    nc.compile()
    return nc


_NC_CACHE = {}


def _get_nc():
    if "nc" not in _NC_CACHE:
        _NC_CACHE["nc"] = build_nc()
    return _NC_CACHE["nc"]


def make_in_maps(inputs, bpc=BPC, ncores=NCORES):
    f = lambda x: np.ascontiguousarray(np.asarray(x, dtype=np.float32))
    shared = {
        "W_h": f(inputs["W_h"]),
        "b_h": f(inputs["b_h"]).reshape(1, D),
        "W_s": f(inputs["W_s"]),
        "b_s": f(inputs["b_s"]).reshape(1, D),
        "W_cov": f(inputs["W_cov"]).reshape(1, D),
        "b_cov": f(inputs["b_cov"]).reshape(1, D),
        "v_w": f(inputs["v_w"]).reshape(1, D),
    }
    mask = np.ascontiguousarray(np.asarray(inputs["context_mask"], dtype=np.int32))
    in_maps = []
    for c in range(ncores):
        sl = slice(c * bpc, (c + 1) * bpc)
        m = dict(shared)
        m["context"] = f(inputs["context"][sl])
        m["state_h"] = f(inputs["state_h"][sl])
        m["state_c"] = f(inputs["state_c"][sl])
        m["coverage"] = f(inputs["coverage"][sl])
        m["context_mask"] = mask[sl]
        in_maps.append(m)
    return in_maps


def gather_outputs(results):
    attn_h = np.concatenate([np.asarray(r["attn_h"]) for r in results], axis=0)
    attn = np.concatenate([np.asarray(r["attn"]) for r in results], axis=0)
    return attn_h.astype(np.float32), attn.astype(np.float32)


def kernel(**inputs):
    from concourse.bass_utils import run_bass_kernel_spmd

    nc = _get_nc()
    in_maps = make_in_maps(inputs)
    res = run_bass_kernel_spmd(nc, in_maps, core_ids=list(range(NCORES)))
    return gather_outputs(res.results)


# revision 25
# speedup vs baseline: 1.5852x; 1.2895x over previous
"""Coverage-attention kernel for Trainium2, data-parallel over batch on 8 NeuronCores.

Reference computation (per batch b):
    cat   = [state_h; state_c]                      (2D,)
    cov   = (coverage + 1e-3) / sum(coverage + 1e-3)
    A     = tanh(context @ W_h + cat @ W_s + cov[:,None]*W_cov + biases)   (S, D)
    e     = A @ v_w                                  (S,)
    attn  = exp(e)*mask / sum(exp(e)*mask)           (softmax shift cancels)
    attn_h = attn @ context                          (D,)

Sharding: batch 32 -> 4 per core; weights replicated. Heavy math in bf16 on
TensorE; fp32 accumulation in PSUM.
"""

import os
import sys

import numpy as np

for _p in ("/opt/trn_rl_repo", "/root/.axon_site/_ro/trn_rl_repo"):
    if os.path.isdir(_p) and _p not in sys.path:
        sys.path.append(_p)

B, S, D = 32, 2048, 1024
NCORES = 8
BPC = B // NCORES          # batches per core
P = 128                    # partitions
KC = D // P                # 8 contraction chunks for the D x D matmul
NT = S // P                # 16 sequence tiles per batch
NH = 512                   # matmul moving free-dim (one PSUM bank of fp32)


def build_nc(bpc=BPC, nt=NT):
    """Build the per-core Bass graph. Identical on all cores (pure SPMD)."""
    import concourse.bass as bass  # noqa: F401
    import concourse.tile as tile
    from concourse import bacc, mybir
    from concourse.masks import make_identity
    from contextlib import ExitStack

    fp32 = mybir.dt.float32
    bf16 = mybir.dt.bfloat16
    i32 = mybir.dt.int32
    s = nt * P
    Tanh = mybir.ActivationFunctionType.Tanh
    Exp = mybir.ActivationFunctionType.Exp
    mult = mybir.AluOpType.mult
    add = mybir.AluOpType.add

    nc = bacc.Bacc(None, target_bir_lowering=False, debug=False)

    ctx_in = nc.declare_dram_parameter("context", [bpc, s, D], fp32, isOutput=False)
    sh_in = nc.declare_dram_parameter("state_h", [bpc, D], fp32, isOutput=False)
    sc_in = nc.declare_dram_parameter("state_c", [bpc, D], fp32, isOutput=False)
    mask_in = nc.declare_dram_parameter("context_mask", [bpc, s], i32, isOutput=False)
    cov_in = nc.declare_dram_parameter("coverage", [bpc, s], fp32, isOutput=False)
    wh_in = nc.declare_dram_parameter("W_h", [D, D], fp32, isOutput=False)
    bh_in = nc.declare_dram_parameter("b_h", [1, D], fp32, isOutput=False)
    ws_in = nc.declare_dram_parameter("W_s", [2 * D, D], fp32, isOutput=False)
    bs_in = nc.declare_dram_parameter("b_s", [1, D], fp32, isOutput=False)
    wcov_in = nc.declare_dram_parameter("W_cov", [1, D], fp32, isOutput=False)
    bcov_in = nc.declare_dram_parameter("b_cov", [1, D], fp32, isOutput=False)
    vw_in = nc.declare_dram_parameter("v_w", [1, D], fp32, isOutput=False)

    ah_out = nc.declare_dram_parameter("attn_h", [bpc, D], fp32, isOutput=True)
    attn_out = nc.declare_dram_parameter("attn", [bpc, s], fp32, isOutput=True)

    # DRAM views with the sequence dim split (t p): p fastest -> contiguous
    mask_v = mask_in[:].rearrange("b (t p) -> b t p", p=P)
    attn_v = attn_out[:].rearrange("b (t p) -> b t p", p=P)
    wh_v = wh_in[:].rearrange("(k p) n -> p k n", p=P)

    with tile.TileContext(nc) as tc, ExitStack() as ctx:
        consts = ctx.enter_context(tc.tile_pool(name="consts", bufs=1))
        ctxp = ctx.enter_context(tc.tile_pool(name="ctxnat", bufs=12))
        trp = ctx.enter_context(tc.tile_pool(name="ctxT", bufs=3))
        actp = ctx.enter_context(tc.tile_pool(name="acts", bufs=2))
        scr = ctx.enter_context(tc.tile_pool(name="scratch", bufs=2))
        bpool = ctx.enter_context(tc.tile_pool(name="perbatch", bufs=2))
        outp = ctx.enter_context(tc.tile_pool(name="outs", bufs=2))
        ps_main = ctx.enter_context(tc.tile_pool(name="psmain", bufs=2, space="PSUM"))
        ps_tr = ctx.enter_context(tc.tile_pool(name="pstr", bufs=2, space="PSUM"))
        ps_attn = ctx.enter_context(tc.tile_pool(name="psattn", bufs=1, space="PSUM"))

        # ---- constants -------------------------------------------------
        ident = consts.tile([P, P], bf16, tag="ident")
        make_identity(nc, ident)

        whb = consts.tile([P, KC, D], bf16, tag="whb")
        nc.gpsimd.dma_start(out=whb, in_=wh_v)

        vw_b = consts.tile([P, D], bf16, tag="vwb")
        nc.gpsimd.dma_start(out=vw_b, in_=vw_in[:].to_broadcast([P, D]))

        ones_col = consts.tile([P, 1], bf16, tag="ones")
        nc.vector.memset(ones_col, 1.0)
        ones_row = consts.tile([1, P], fp32, tag="onesrow")
        nc.vector.memset(ones_row, 1.0)

        # persistent results of the prologue, consumed per batch
        wstate_b = consts.tile([bpc, D], bf16, tag="wstateb")
        cov_nb = consts.tile([bpc, s], bf16, tag="covnb")

        # prologue-only tiles live in a pool that closes before the main
        # loop so their SBUF space is recycled
        with tc.tile_pool(name="prologue", bufs=1) as pro, \
             tc.tile_pool(name="wstream", bufs=2) as wspool:
            # w_state = cat(state_h, state_c) @ W_s + (b_s + b_h + b_cov)
            cat_f = pro.tile([bpc, 2 * D], fp32, tag="catf")
            nc.sync.dma_start(out=cat_f[:, 0:D], in_=sh_in[:])
            nc.sync.dma_start(out=cat_f[:, D : 2 * D], in_=sc_in[:])
            cat_b = pro.tile([bpc, 2 * D], bf16, tag="catb")
            nc.vector.tensor_copy(cat_b, cat_f)

            cstT = pro.tile([P, 2 * KC, bpc], bf16, tag="cstT")
            for k in range(2 * KC):
                pt = ps_tr.tile([P, bpc], bf16, tag="tr")
                nc.tensor.transpose(pt, cat_b[:, k * P : (k + 1) * P], ident[:bpc, :bpc])
                nc.vector.tensor_copy(cstT[:, k, :], pt)

            psw = ps_main.tile([bpc, D], fp32, tag="main")
            for k in range(2 * KC):
                wst = wspool.tile([P, D], bf16, tag="ws")
                nc.gpsimd.dma_start(out=wst, in_=ws_in[k * P : (k + 1) * P, :])
                for n in range(D // NH):
                    nc.tensor.matmul(
                        psw[:, n * NH : (n + 1) * NH],
                        lhsT=cstT[:, k, :],
                        rhs=wst[:, n * NH : (n + 1) * NH],
                        start=(k == 0),
                        stop=(k == 2 * KC - 1),
                    )
            bias_t = pro.tile([bpc, D], fp32, tag="biast")
            nc.gpsimd.dma_start(out=bias_t, in_=bs_in[:].to_broadcast([bpc, D]))
            tmp_b = pro.tile([bpc, D], fp32, tag="biastmp")
            nc.gpsimd.dma_start(out=tmp_b, in_=bh_in[:].to_broadcast([bpc, D]))
            nc.vector.tensor_add(bias_t, bias_t, tmp_b)
            tmp_b2 = pro.tile([bpc, D], fp32, tag="biastmp")
            nc.gpsimd.dma_start(out=tmp_b2, in_=bcov_in[:].to_broadcast([bpc, D]))
            nc.vector.tensor_add(bias_t, bias_t, tmp_b2)
            wstate = pro.tile([bpc, D], fp32, tag="wstate")
            nc.vector.tensor_add(wstate, psw, bias_t)
            nc.vector.tensor_copy(wstate_b, wstate)

            # coverage normalization
            cov_f = pro.tile([bpc, s], fp32, tag="covf")
            nc.sync.dma_start(out=cov_f, in_=cov_in[:])
            cov_sum = pro.tile([bpc, 1], fp32, tag="covsum")
            nc.vector.tensor_reduce(out=cov_sum, in_=cov_f, axis=mybir.AxisListType.X, op=add)
            nc.vector.tensor_scalar_add(cov_sum, cov_sum, 0.001 * s)
            cov_rcp = pro.tile([bpc, 1], fp32, tag="covrcp")
            nc.vector.reciprocal(cov_rcp, cov_sum)
            nc.vector.tensor_scalar(
                out=cov_nb, in0=cov_f, scalar1=0.001, scalar2=cov_rcp, op0=add, op1=mult
            )

        # ---- main loop --------------------------------------------------
        for b in range(bpc):
            # rank-1 bias operands for this batch
            r1b = bpool.tile([2, D], bf16, tag="r1b")
            nc.gpsimd.dma_start(out=r1b[0:1, :], in_=wstate_b[b : b + 1, :])
            nc.gpsimd.dma_start(out=r1b[1:2, :], in_=wcov_in[:])
            lcovb = bpool.tile([2, s], bf16, tag="lcovb")
            nc.vector.memset(lcovb[0:1, :], 1.0)
            nc.sync.dma_start(out=lcovb[1:2, :], in_=cov_nb[b : b + 1, :])
            # mask -> (P, nt) bf16 for this batch
            mi = bpool.tile([nt, P], i32, tag="maski")
            nc.sync.dma_start(out=mi, in_=mask_v[b])
            mf = bpool.tile([nt, P], fp32, tag="maskf")
            nc.vector.tensor_copy(mf, mi)
            mb = bpool.tile([nt, P], bf16, tag="maskb")
            nc.vector.tensor_copy(mb, mf)
            pmt = ps_tr.tile([P, nt], bf16, tag="tr")
            nc.tensor.transpose(pmt, mb, ident[:nt, :nt])
            mask_t = bpool.tile([P, nt], bf16, tag="maskt")
            nc.vector.tensor_copy(mask_t, pmt)

            # Per-tile: load context bf16, transpose 128x128 blocks on the PE
            # (software-pipelined one tile ahead so the PSUM->SBUF copies never
            # stall the PE), then GEMMs + fused softmax-numerator + attn_h
            # accumulation. No max-subtraction is needed (|e| is small and the
            # shift cancels), so p = exp(e)*mask accumulates into attn_h
            # immediately and the context tile is released after ~2 tiles.
            p_buf = bpool.tile([P, nt], bf16, tag="pbuf")
            pa = ps_attn.tile([1, D], fp32, tag="attn")
            nat_tiles = [None] * nt
            ctxT_tiles = [None] * nt

            def emit_T(t):
                cn = ctxp.tile([P, D], bf16, tag="nat")
                nat_tiles[t] = cn
                nc.gpsimd.dma_start(out=cn, in_=ctx_in[b, t * P : (t + 1) * P, :])
                ptr = ps_tr.tile([P, KC, P], bf16, tag="tr")
                ctxTt = trp.tile([P, KC, P], bf16, tag="ctxT")
                ctxT_tiles[t] = ctxTt
                for k in range(KC):
                    nc.tensor.transpose(ptr[:, k, :], cn[:, k * P : (k + 1) * P], ident)
                    if k % 2 == 0:
                        nc.vector.tensor_copy(ctxTt[:, k, :], ptr[:, k, :])
                    else:
                        nc.scalar.copy(ctxTt[:, k, :], ptr[:, k, :])

            def emit_M(t):
                pm = ps_main.tile([P, D], fp32, tag="main")
                for n in range(D // NH):
                    nc.tensor.matmul(
                        pm[:, n * NH : (n + 1) * NH],
                        lhsT=lcovb[:, t * P : (t + 1) * P],
                        rhs=r1b[:, n * NH : (n + 1) * NH],
                        start=True,
                        stop=False,
                    )
                for k in range(KC):
                    for n in range(D // NH):
                        nc.tensor.matmul(
                            pm[:, n * NH : (n + 1) * NH],
                            lhsT=ctxT_tiles[t][:, k, :],
                            rhs=whb[:, k, n * NH : (n + 1) * NH],
                            start=False,
                            stop=(k == KC - 1),
                        )
                act = actp.tile([P, D], bf16, tag="act")
                nc.scalar.activation(act, pm, Tanh)
                ttro = scr.tile([P, D], bf16, tag="ttro")
                nc.vector.tensor_mul(ttro, act, vw_b)
                e_col = scr.tile([P, 1], fp32, tag="ecol")
                nc.vector.tensor_reduce(
                    out=e_col, in_=ttro, axis=mybir.AxisListType.X, op=add
                )
                nc.scalar.activation(p_buf[:, t : t + 1], e_col, Exp)
                nc.vector.tensor_mul(
                    p_buf[:, t : t + 1], p_buf[:, t : t + 1], mask_t[:, t : t + 1]
                )
                for tp in ([t - 1] if t >= 1 else []) + ([t] if t == nt - 1 else []):
                    for n in range(D // NH):
                        nc.tensor.matmul(
                            pa[:, n * NH : (n + 1) * NH],
                            lhsT=p_buf[:, tp : tp + 1],
                            rhs=nat_tiles[tp][:, n * NH : (n + 1) * NH],
                            start=(tp == 0),
                            stop=(tp == nt - 1),
                        )

            emit_T(0)
            for t in range(nt):
                if t + 1 < nt:
                    emit_T(t + 1)
                emit_M(t)

            # Z = sum over all positions (partition-sum via ones matmul)
            zp = ps_tr.tile([1, nt], fp32, tag="tr")
            nc.tensor.matmul(zp, lhsT=ones_col, rhs=p_buf, start=True, stop=True)
            zs = outp.tile([1, 1], fp32, tag="zs")
            nc.vector.tensor_reduce(out=zs, in_=zp, axis=mybir.AxisListType.X, op=add)
            # broadcast Z to all partitions via a K=1 ones matmul, then 1/Z
            zbp = ps_tr.tile([P, 1], fp32, tag="tr")
            nc.tensor.matmul(zbp, lhsT=ones_row, rhs=zs, start=True, stop=True)
            rzb = outp.tile([P, 1], fp32, tag="rzb")
            nc.vector.reciprocal(rzb, zbp)
            ah = outp.tile([1, D], fp32, tag="ah")
            nc.vector.tensor_scalar_mul(ah, pa, rzb[0:1, :])
            nc.sync.dma_start(out=ah_out[b : b + 1, :], in_=ah)

            # attn row out: scale (bf16), transpose to (t, p) so DRAM writes are
            # contiguous, cast to fp32 on the PSUM->SBUF copy
            arow = outp.tile([P, nt], bf16, tag="arow")
            nc.vector.tensor_scalar_mul(arow, p_buf, rzb)
            pat = ps_tr.tile([nt, P], bf16, tag="tr")
            nc.tensor.transpose(pat, arow, ident)
            arow_t = outp.tile([nt, P], fp32, tag="arowt")
            nc.vector.tensor_copy(arow_t, pat)
            nc.sync.dma_start(out=attn_v[b], in_=arow_t)

    nc.compile()
    return nc


_NC_CACHE = {}


def _get_nc():
    if "nc" not in _NC_CACHE:
        _NC_CACHE["nc"] = build_nc()
    return _NC_CACHE["nc"]


def make_in_maps(inputs, bpc=BPC, ncores=NCORES):
    f = lambda x: np.ascontiguousarray(np.asarray(x, dtype=np.float32))
    shared = {
        "W_h": f(inputs["W_h"]),
        "b_h": f(inputs["b_h"]).reshape(1, D),
        "W_s": f(inputs["W_s"]),
        "b_s": f(inputs["b_s"]).reshape(1, D),
        "W_cov": f(inputs["W_cov"]).reshape(1, D),
        "b_cov": f(inputs["b_cov"]).reshape(1, D),
        "v_w": f(inputs["v_w"]).reshape(1, D),
    }
    mask = np.ascontiguousarray(np.asarray(inputs["context_mask"], dtype=np.int32))
    in_maps = []
    for c in range(ncores):
        sl = slice(c * bpc, (c + 1) * bpc)
        m = dict(shared)
        m["context"] = f(inputs["context"][sl])
        m["state_h"] = f(inputs["state_h"][sl])
        m["state_c"] = f(inputs["state_c"][sl])
        m["coverage"] = f(inputs["coverage"][sl])
        m["context_mask"] = mask[sl]
        in_maps.append(m)
    return in_maps


def gather_outputs(results):
    attn_h = np.concatenate([np.asarray(r["attn_h"]) for r in results], axis=0)
    attn = np.concatenate([np.asarray(r["attn"]) for r in results], axis=0)
    return attn_h.astype(np.float32), attn.astype(np.float32)


def kernel(**inputs):
    from concourse.bass_utils import run_bass_kernel_spmd

    nc = _get_nc()
    in_maps = make_in_maps(inputs)
    res = run_bass_kernel_spmd(nc, in_maps, core_ids=list(range(NCORES)))
    return gather_outputs(res.results)


# revision 26
# speedup vs baseline: 1.6427x; 1.0362x over previous
"""Coverage-attention kernel for Trainium2, data-parallel over batch on 8 NeuronCores.

Reference computation (per batch b):
    cat   = [state_h; state_c]                      (2D,)
    cov   = (coverage + 1e-3) / sum(coverage + 1e-3)
    A     = tanh(context @ W_h + cat @ W_s + cov[:,None]*W_cov + biases)   (S, D)
    e     = A @ v_w                                  (S,)
    attn  = exp(e)*mask / sum(exp(e)*mask)           (softmax shift cancels)
    attn_h = attn @ context                          (D,)

Sharding: batch 32 -> 4 per core; weights replicated. Heavy math in bf16 on
TensorE; fp32 accumulation in PSUM.
"""

import os
import sys

import numpy as np

for _p in ("/opt/trn_rl_repo", "/root/.axon_site/_ro/trn_rl_repo"):
    if os.path.isdir(_p) and _p not in sys.path:
        sys.path.append(_p)

B, S, D = 32, 2048, 1024
NCORES = 8
BPC = B // NCORES          # batches per core
P = 128                    # partitions
KC = D // P                # 8 contraction chunks for the D x D matmul
NT = S // P                # 16 sequence tiles per batch
NH = 512                   # matmul moving free-dim (one PSUM bank of fp32)


def build_nc(bpc=BPC, nt=NT):
    """Build the per-core Bass graph. Identical on all cores (pure SPMD)."""
    import concourse.bass as bass  # noqa: F401
    import concourse.tile as tile
    from concourse import bacc, mybir
    from concourse.masks import make_identity
    from contextlib import ExitStack

    fp32 = mybir.dt.float32
    bf16 = mybir.dt.bfloat16
    i32 = mybir.dt.int32
    s = nt * P
    Tanh = mybir.ActivationFunctionType.Tanh
    Exp = mybir.ActivationFunctionType.Exp
    mult = mybir.AluOpType.mult
    add = mybir.AluOpType.add

    nc = bacc.Bacc(None, target_bir_lowering=False, debug=False)

    ctx_in = nc.declare_dram_parameter("context", [bpc, s, D], fp32, isOutput=False)
    sh_in = nc.declare_dram_parameter("state_h", [bpc, D], fp32, isOutput=False)
    sc_in = nc.declare_dram_parameter("state_c", [bpc, D], fp32, isOutput=False)
    mask_in = nc.declare_dram_parameter("context_mask", [bpc, s], i32, isOutput=False)
    cov_in = nc.declare_dram_parameter("coverage", [bpc, s], fp32, isOutput=False)
    wh_in = nc.declare_dram_parameter("W_h", [D, D], fp32, isOutput=False)
    bh_in = nc.declare_dram_parameter("b_h", [1, D], fp32, isOutput=False)
    ws_in = nc.declare_dram_parameter("W_s", [2 * D, D], fp32, isOutput=False)
    bs_in = nc.declare_dram_parameter("b_s", [1, D], fp32, isOutput=False)
    wcov_in = nc.declare_dram_parameter("W_cov", [1, D], fp32, isOutput=False)
    bcov_in = nc.declare_dram_parameter("b_cov", [1, D], fp32, isOutput=False)
    vw_in = nc.declare_dram_parameter("v_w", [1, D], fp32, isOutput=False)

    ah_out = nc.declare_dram_parameter("attn_h", [bpc, D], fp32, isOutput=True)
    attn_out = nc.declare_dram_parameter("attn", [bpc, s], fp32, isOutput=True)

    # DRAM views with the sequence dim split (t p): p fastest -> contiguous
    mask_v = mask_in[:].rearrange("b (t p) -> b t p", p=P)
    attn_v = attn_out[:].rearrange("b (t p) -> b t p", p=P)
    wh_v = wh_in[:].rearrange("(k p) n -> p k n", p=P)

    with tile.TileContext(nc) as tc, ExitStack() as ctx:
        consts = ctx.enter_context(tc.tile_pool(name="consts", bufs=1))
        ctxp = ctx.enter_context(tc.tile_pool(name="ctxnat", bufs=12))
        trp = ctx.enter_context(tc.tile_pool(name="ctxT", bufs=3))
        actp = ctx.enter_context(tc.tile_pool(name="acts", bufs=2))
        scr = ctx.enter_context(tc.tile_pool(name="scratch", bufs=2))
        bpool = ctx.enter_context(tc.tile_pool(name="perbatch", bufs=2))
        outp = ctx.enter_context(tc.tile_pool(name="outs", bufs=2))
        ps_main = ctx.enter_context(tc.tile_pool(name="psmain", bufs=2, space="PSUM"))
        ps_tr = ctx.enter_context(tc.tile_pool(name="pstr", bufs=2, space="PSUM"))
        ps_attn = ctx.enter_context(tc.tile_pool(name="psattn", bufs=1, space="PSUM"))

        # ---- constants -------------------------------------------------
        ident = consts.tile([P, P], bf16, tag="ident")
        make_identity(nc, ident)

        whb = consts.tile([P, KC, D], bf16, tag="whb")
        nc.gpsimd.dma_start(out=whb, in_=wh_v)

        vw_b = consts.tile([P, D], bf16, tag="vwb")
        nc.gpsimd.dma_start(out=vw_b, in_=vw_in[:].to_broadcast([P, D]))

        ones_col = consts.tile([P, 1], bf16, tag="ones")
        nc.vector.memset(ones_col, 1.0)
        ones_row = consts.tile([1, P], fp32, tag="onesrow")
        nc.vector.memset(ones_row, 1.0)

        # persistent results of the prologue, consumed per batch
        wstate_b = consts.tile([bpc, D], bf16, tag="wstateb")
        cov_nb = consts.tile([bpc, s], bf16, tag="covnb")
        wcov_bf = consts.tile([1, D], bf16, tag="wcovbf")
        nc.gpsimd.dma_start(out=wcov_bf, in_=wcov_in[:])

        # prologue-only tiles live in a pool that closes before the main
        # loop so their SBUF space is recycled
        with tc.tile_pool(name="prologue", bufs=1) as pro, \
             tc.tile_pool(name="wstream", bufs=2) as wspool:
            # w_state = cat(state_h, state_c) @ W_s + (b_s + b_h + b_cov)
            cat_f = pro.tile([bpc, 2 * D], fp32, tag="catf")
            nc.sync.dma_start(out=cat_f[:, 0:D], in_=sh_in[:])
            nc.sync.dma_start(out=cat_f[:, D : 2 * D], in_=sc_in[:])
            cat_b = pro.tile([bpc, 2 * D], bf16, tag="catb")
            nc.vector.tensor_copy(cat_b, cat_f)

            cstT = pro.tile([P, 2 * KC, bpc], bf16, tag="cstT")
            for k in range(2 * KC):
                pt = ps_tr.tile([P, bpc], bf16, tag="tr")
                nc.tensor.transpose(pt, cat_b[:, k * P : (k + 1) * P], ident[:bpc, :bpc])
                nc.vector.tensor_copy(cstT[:, k, :], pt)

            psw = ps_main.tile([bpc, D], fp32, tag="main")
            for k in range(2 * KC):
                wst = wspool.tile([P, D], bf16, tag="ws")
                nc.gpsimd.dma_start(out=wst, in_=ws_in[k * P : (k + 1) * P, :])
                for n in range(D // NH):
                    nc.tensor.matmul(
                        psw[:, n * NH : (n + 1) * NH],
                        lhsT=cstT[:, k, :],
                        rhs=wst[:, n * NH : (n + 1) * NH],
                        start=(k == 0),
                        stop=(k == 2 * KC - 1),
                    )
            bias_t = pro.tile([bpc, D], fp32, tag="biast")
            nc.gpsimd.dma_start(out=bias_t, in_=bs_in[:].to_broadcast([bpc, D]))
            tmp_b = pro.tile([bpc, D], fp32, tag="biastmp")
            nc.gpsimd.dma_start(out=tmp_b, in_=bh_in[:].to_broadcast([bpc, D]))
            nc.vector.tensor_add(bias_t, bias_t, tmp_b)
            tmp_b2 = pro.tile([bpc, D], fp32, tag="biastmp")
            nc.gpsimd.dma_start(out=tmp_b2, in_=bcov_in[:].to_broadcast([bpc, D]))
            nc.vector.tensor_add(bias_t, bias_t, tmp_b2)
            wstate = pro.tile([bpc, D], fp32, tag="wstate")
            nc.vector.tensor_add(wstate, psw, bias_t)
            nc.vector.tensor_copy(wstate_b, wstate)

            # coverage normalization
            cov_f = pro.tile([bpc, s], fp32, tag="covf")
            nc.sync.dma_start(out=cov_f, in_=cov_in[:])
            cov_sum = pro.tile([bpc, 1], fp32, tag="covsum")
            nc.vector.tensor_reduce(out=cov_sum, in_=cov_f, axis=mybir.AxisListType.X, op=add)
            nc.vector.tensor_scalar_add(cov_sum, cov_sum, 0.001 * s)
            cov_rcp = pro.tile([bpc, 1], fp32, tag="covrcp")
            nc.vector.reciprocal(cov_rcp, cov_sum)
            nc.vector.tensor_scalar(
                out=cov_nb, in0=cov_f, scalar1=0.001, scalar2=cov_rcp, op0=add, op1=mult
            )

        # ---- main loop --------------------------------------------------
        pending_epilogue = [None]
        for b in range(bpc):
            # rank-1 bias operands for this batch
            r1b = bpool.tile([2, D], bf16, tag="r1b")
            nc.sync.dma_start(out=r1b[0:1, :], in_=wstate_b[b : b + 1, :])
            nc.sync.dma_start(out=r1b[1:2, :], in_=wcov_bf)
            lcovb = bpool.tile([2, s], bf16, tag="lcovb")
            nc.vector.memset(lcovb[0:1, :], 1.0)
            nc.sync.dma_start(out=lcovb[1:2, :], in_=cov_nb[b : b + 1, :])
            # mask -> (P, nt) bf16 for this batch
            mi = bpool.tile([nt, P], i32, tag="maski")
            nc.sync.dma_start(out=mi, in_=mask_v[b])
            mf = bpool.tile([nt, P], fp32, tag="maskf")
            nc.vector.tensor_copy(mf, mi)
            mb = bpool.tile([nt, P], bf16, tag="maskb")
            nc.vector.tensor_copy(mb, mf)
            pmt = ps_tr.tile([P, nt], bf16, tag="tr")
            nc.tensor.transpose(pmt, mb, ident[:nt, :nt])
            mask_t = bpool.tile([P, nt], bf16, tag="maskt")
            nc.vector.tensor_copy(mask_t, pmt)

            # Per-tile: load context bf16, transpose 128x128 blocks on the PE
            # (software-pipelined one tile ahead so the PSUM->SBUF copies never
            # stall the PE), then GEMMs + fused softmax-numerator + attn_h
            # accumulation. No max-subtraction is needed (|e| is small and the
            # shift cancels), so p = exp(e)*mask accumulates into attn_h
            # immediately and the context tile is released after ~2 tiles.
            p_buf = bpool.tile([P, nt], bf16, tag="pbuf")
            pa = ps_attn.tile([1, D], fp32, tag="attn")
            nat_tiles = [None] * nt
            ctxT_tiles = [None] * nt

            def emit_T(t):
                cn = ctxp.tile([P, D], bf16, tag="nat")
                nat_tiles[t] = cn
                nc.gpsimd.dma_start(out=cn, in_=ctx_in[b, t * P : (t + 1) * P, :])
                ptr = ps_tr.tile([P, KC, P], bf16, tag="tr")
                ctxTt = trp.tile([P, KC, P], bf16, tag="ctxT")
                ctxT_tiles[t] = ctxTt
                for k in range(KC):
                    nc.tensor.transpose(ptr[:, k, :], cn[:, k * P : (k + 1) * P], ident)
                    if k % 2 == 0:
                        nc.vector.tensor_copy(ctxTt[:, k, :], ptr[:, k, :])
                    else:
                        nc.scalar.copy(ctxTt[:, k, :], ptr[:, k, :])

            def emit_M(t):
                pm = ps_main.tile([P, D], fp32, tag="main")
                for n in range(D // NH):
                    nc.tensor.matmul(
                        pm[:, n * NH : (n + 1) * NH],
                        lhsT=lcovb[:, t * P : (t + 1) * P],
                        rhs=r1b[:, n * NH : (n + 1) * NH],
                        start=True,
                        stop=False,
                    )
                for k in range(KC):
                    for n in range(D // NH):
                        nc.tensor.matmul(
                            pm[:, n * NH : (n + 1) * NH],
                            lhsT=ctxT_tiles[t][:, k, :],
                            rhs=whb[:, k, n * NH : (n + 1) * NH],
                            start=False,
                            stop=(k == KC - 1),
                        )
                act = actp.tile([P, D], bf16, tag="act")
                nc.scalar.activation(act, pm, Tanh)
                ttro = scr.tile([P, D], bf16, tag="ttro")
                nc.vector.tensor_mul(ttro, act, vw_b)
                e_col = scr.tile([P, 1], fp32, tag="ecol")
                nc.vector.tensor_reduce(
                    out=e_col, in_=ttro, axis=mybir.AxisListType.X, op=add
                )
                nc.scalar.activation(p_buf[:, t : t + 1], e_col, Exp)
                nc.vector.tensor_mul(
                    p_buf[:, t : t + 1], p_buf[:, t : t + 1], mask_t[:, t : t + 1]
                )
                for tp in ([t - 1] if t >= 1 else []) + ([t] if t == nt - 1 else []):
                    for n in range(D // NH):
                        nc.tensor.matmul(
                            pa[:, n * NH : (n + 1) * NH],
                            lhsT=p_buf[:, tp : tp + 1],
                            rhs=nat_tiles[tp][:, n * NH : (n + 1) * NH],
                            start=(tp == 0),
                            stop=(tp == nt - 1),
                        )

            def make_epilogue(b=b, p_buf=p_buf, pa=pa):
                def epilogue():
                    # Z = sum over all positions (partition-sum via ones matmul)
                    zp = ps_tr.tile([1, nt], fp32, tag="tr")
                    nc.tensor.matmul(zp, lhsT=ones_col, rhs=p_buf, start=True, stop=True)
                    zs = outp.tile([1, 1], fp32, tag="zs")
                    nc.vector.tensor_reduce(
                        out=zs, in_=zp, axis=mybir.AxisListType.X, op=add
                    )
                    # broadcast Z to all partitions via a K=1 ones matmul, then 1/Z
                    zbp = ps_tr.tile([P, 1], fp32, tag="tr")
                    nc.tensor.matmul(zbp, lhsT=ones_row, rhs=zs, start=True, stop=True)
                    rzb = outp.tile([P, 1], fp32, tag="rzb")
                    nc.vector.reciprocal(rzb, zbp)
                    ah = outp.tile([1, D], fp32, tag="ah")
                    nc.vector.tensor_scalar_mul(ah, pa, rzb[0:1, :])
                    nc.sync.dma_start(out=ah_out[b : b + 1, :], in_=ah)
                    # attn row: scale (bf16), transpose to (t, p) for a
                    # contiguous DRAM write, cast fp32 on the PSUM->SBUF copy
                    arow = outp.tile([P, nt], bf16, tag="arow")
                    nc.vector.tensor_scalar_mul(arow, p_buf, rzb)
                    pat = ps_tr.tile([nt, P], bf16, tag="tr")
                    nc.tensor.transpose(pat, arow, ident)
                    arow_t = outp.tile([nt, P], fp32, tag="arowt")
                    nc.vector.tensor_copy(arow_t, pat)
                    nc.sync.dma_start(out=attn_v[b], in_=arow_t)
                return epilogue

            emit_T(0)
            for t in range(nt):
                if t + 1 < nt:
                    emit_T(t + 1)
                emit_M(t)
                if t == 0 and pending_epilogue[0] is not None:
                    pending_epilogue[0]()
                    pending_epilogue[0] = None
            pending_epilogue[0] = make_epilogue()

        pending_epilogue[0]()

    nc.compile()
    return nc


_NC_CACHE = {}


def _get_nc():
    if "nc" not in _NC_CACHE:
        _NC_CACHE["nc"] = build_nc()
    return _NC_CACHE["nc"]


def make_in_maps(inputs, bpc=BPC, ncores=NCORES):
    f = lambda x: np.ascontiguousarray(np.asarray(x, dtype=np.float32))
    shared = {
        "W_h": f(inputs["W_h"]),
        "b_h": f(inputs["b_h"]).reshape(1, D),
        "W_s": f(inputs["W_s"]),
        "b_s": f(inputs["b_s"]).reshape(1, D),
        "W_cov": f(inputs["W_cov"]).reshape(1, D),
        "b_cov": f(inputs["b_cov"]).reshape(1, D),
        "v_w": f(inputs["v_w"]).reshape(1, D),
    }
    mask = np.ascontiguousarray(np.asarray(inputs["context_mask"], dtype=np.int32))
    in_maps = []
    for c in range(ncores):
        sl = slice(c * bpc, (c + 1) * bpc)
        m = dict(shared)
        m["context"] = f(inputs["context"][sl])
        m["state_h"] = f(inputs["state_h"][sl])
        m["state_c"] = f(inputs["state_c"][sl])
        m["coverage"] = f(inputs["coverage"][sl])
        m["context_mask"] = mask[sl]
        in_maps.append(m)
    return in_maps


def gather_outputs(results):
    attn_h = np.concatenate([np.asarray(r["attn_h"]) for r in results], axis=0)
    attn = np.concatenate([np.asarray(r["attn"]) for r in results], axis=0)
    return attn_h.astype(np.float32), attn.astype(np.float32)


def kernel(**inputs):
    from concourse.bass_utils import run_bass_kernel_spmd

    nc = _get_nc()
    in_maps = make_in_maps(inputs)
    res = run_bass_kernel_spmd(nc, in_maps, core_ids=list(range(NCORES)))
    return gather_outputs(res.results)


# revision 27
# speedup vs baseline: 1.6637x; 1.0128x over previous
"""Coverage-attention kernel for Trainium2, data-parallel over batch on 8 NeuronCores.

Reference computation (per batch b):
    cat   = [state_h; state_c]                      (2D,)
    cov   = (coverage + 1e-3) / sum(coverage + 1e-3)
    A     = tanh(context @ W_h + cat @ W_s + cov[:,None]*W_cov + biases)   (S, D)
    e     = A @ v_w                                  (S,)
    attn  = exp(e)*mask / sum(exp(e)*mask)           (softmax shift cancels)
    attn_h = attn @ context                          (D,)

Sharding: batch 32 -> 4 per core; weights replicated. Heavy math in bf16 on
TensorE; fp32 accumulation in PSUM.
"""

import os
import sys

import numpy as np

for _p in ("/opt/trn_rl_repo", "/root/.axon_site/_ro/trn_rl_repo"):
    if os.path.isdir(_p) and _p not in sys.path:
        sys.path.append(_p)

B, S, D = 32, 2048, 1024
NCORES = 8
BPC = B // NCORES          # batches per core
P = 128                    # partitions
KC = D // P                # 8 contraction chunks for the D x D matmul
NT = S // P                # 16 sequence tiles per batch
NH = 512                   # matmul moving free-dim (one PSUM bank of fp32)


def build_nc(bpc=BPC, nt=NT):
    """Build the per-core Bass graph. Identical on all cores (pure SPMD)."""
    import concourse.bass as bass  # noqa: F401
    import concourse.tile as tile
    from concourse import bacc, mybir
    from concourse.masks import make_identity
    from contextlib import ExitStack

    fp32 = mybir.dt.float32
    bf16 = mybir.dt.bfloat16
    i32 = mybir.dt.int32
    s = nt * P
    Tanh = mybir.ActivationFunctionType.Tanh
    Exp = mybir.ActivationFunctionType.Exp
    mult = mybir.AluOpType.mult
    add = mybir.AluOpType.add

    nc = bacc.Bacc(None, target_bir_lowering=False, debug=False)

    ctx_in = nc.declare_dram_parameter("context", [bpc, s, D], fp32, isOutput=False)
    sh_in = nc.declare_dram_parameter("state_h", [bpc, D], fp32, isOutput=False)
    sc_in = nc.declare_dram_parameter("state_c", [bpc, D], fp32, isOutput=False)
    mask_in = nc.declare_dram_parameter("context_mask", [bpc, s], i32, isOutput=False)
    cov_in = nc.declare_dram_parameter("coverage", [bpc, s], fp32, isOutput=False)
    wh_in = nc.declare_dram_parameter("W_h", [D, D], fp32, isOutput=False)
    bh_in = nc.declare_dram_parameter("b_h", [1, D], fp32, isOutput=False)
    ws_in = nc.declare_dram_parameter("W_s", [2 * D, D], fp32, isOutput=False)
    bs_in = nc.declare_dram_parameter("b_s", [1, D], fp32, isOutput=False)
    wcov_in = nc.declare_dram_parameter("W_cov", [1, D], fp32, isOutput=False)
    bcov_in = nc.declare_dram_parameter("b_cov", [1, D], fp32, isOutput=False)
    vw_in = nc.declare_dram_parameter("v_w", [1, D], fp32, isOutput=False)

    ah_out = nc.declare_dram_parameter("attn_h", [bpc, D], fp32, isOutput=True)
    attn_out = nc.declare_dram_parameter("attn", [bpc, s], fp32, isOutput=True)

    # DRAM views with the sequence dim split (t p): p fastest -> contiguous
    mask_v = mask_in[:].rearrange("b (t p) -> b t p", p=P)
    attn_v = attn_out[:].rearrange("b (t p) -> b t p", p=P)
    wh_v = wh_in[:].rearrange("(k p) n -> p k n", p=P)

    with tile.TileContext(nc) as tc, ExitStack() as ctx:
        consts = ctx.enter_context(tc.tile_pool(name="consts", bufs=1))
        ctxp = ctx.enter_context(tc.tile_pool(name="ctxnat", bufs=12))
        trp = ctx.enter_context(tc.tile_pool(name="ctxT", bufs=3))
        actp = ctx.enter_context(tc.tile_pool(name="acts", bufs=2))
        scr = ctx.enter_context(tc.tile_pool(name="scratch", bufs=2))
        bpool = ctx.enter_context(tc.tile_pool(name="perbatch", bufs=2))
        outp = ctx.enter_context(tc.tile_pool(name="outs", bufs=2))
        ps_main = ctx.enter_context(tc.tile_pool(name="psmain", bufs=2, space="PSUM"))
        ps_tr = ctx.enter_context(tc.tile_pool(name="pstr", bufs=2, space="PSUM"))
        ps_attn = ctx.enter_context(tc.tile_pool(name="psattn", bufs=1, space="PSUM"))

        # ---- constants -------------------------------------------------
        ident = consts.tile([P, P], bf16, tag="ident")
        make_identity(nc, ident)

        # preload the first context tiles of batch 0 so the PE has transpose
        # work immediately; the rest of the ramp streams behind them
        preload = []
        for t in range(2):
            cn0 = ctxp.tile([P, D], bf16, tag="nat")
            nc.gpsimd.dma_start(out=cn0, in_=ctx_in[0, t * P : (t + 1) * P, :])
            preload.append(cn0)

        vw_b = consts.tile([P, D], bf16, tag="vwb")
        nc.gpsimd.dma_start(out=vw_b, in_=vw_in[:].to_broadcast([P, D]))

        ones_col = consts.tile([P, 1], bf16, tag="ones")
        nc.vector.memset(ones_col, 1.0)
        ones_row = consts.tile([1, P], fp32, tag="onesrow")
        nc.vector.memset(ones_row, 1.0)

        # persistent results of the prologue, consumed per batch
        wstate_b = consts.tile([bpc, D], bf16, tag="wstateb")
        cov_nb = consts.tile([bpc, s], bf16, tag="covnb")
        wcov_bf = consts.tile([1, D], bf16, tag="wcovbf")
        nc.gpsimd.dma_start(out=wcov_bf, in_=wcov_in[:])
        whb = consts.tile([P, KC, D], bf16, tag="whb")

        # prologue-only tiles live in a pool that closes before the main
        # loop so their SBUF space is recycled
        with tc.tile_pool(name="prologue", bufs=1) as pro, \
             tc.tile_pool(name="wstream", bufs=2) as wspool:
            # w_state = cat(state_h, state_c) @ W_s + (b_s + b_h + b_cov)
            cat_f = pro.tile([bpc, 2 * D], fp32, tag="catf")
            nc.sync.dma_start(out=cat_f[:, 0:D], in_=sh_in[:])
            nc.sync.dma_start(out=cat_f[:, D : 2 * D], in_=sc_in[:])
            cat_b = pro.tile([bpc, 2 * D], bf16, tag="catb")
            nc.vector.tensor_copy(cat_b, cat_f)

            cstT = pro.tile([P, 2 * KC, bpc], bf16, tag="cstT")
            for k in range(2 * KC):
                pt = ps_tr.tile([P, bpc], bf16, tag="tr")
                nc.tensor.transpose(pt, cat_b[:, k * P : (k + 1) * P], ident[:bpc, :bpc])
                nc.vector.tensor_copy(cstT[:, k, :], pt)

            psw = ps_main.tile([bpc, D], fp32, tag="main")
            for k in range(2 * KC):
                wst = wspool.tile([P, D], bf16, tag="ws")
                nc.gpsimd.dma_start(out=wst, in_=ws_in[k * P : (k + 1) * P, :])
                for n in range(D // NH):
                    nc.tensor.matmul(
                        psw[:, n * NH : (n + 1) * NH],
                        lhsT=cstT[:, k, :],
                        rhs=wst[:, n * NH : (n + 1) * NH],
                        start=(k == 0),
                        stop=(k == 2 * KC - 1),
                    )
            bias_t = pro.tile([bpc, D], fp32, tag="biast")
            nc.gpsimd.dma_start(out=bias_t, in_=bs_in[:].to_broadcast([bpc, D]))
            tmp_b = pro.tile([bpc, D], fp32, tag="biastmp")
            nc.gpsimd.dma_start(out=tmp_b, in_=bh_in[:].to_broadcast([bpc, D]))
            nc.vector.tensor_add(bias_t, bias_t, tmp_b)
            tmp_b2 = pro.tile([bpc, D], fp32, tag="biastmp")
            nc.gpsimd.dma_start(out=tmp_b2, in_=bcov_in[:].to_broadcast([bpc, D]))
            nc.vector.tensor_add(bias_t, bias_t, tmp_b2)
            wstate = pro.tile([bpc, D], fp32, tag="wstate")
            nc.vector.tensor_add(wstate, psw, bias_t)
            nc.vector.tensor_copy(wstate_b, wstate)

            # W_h streams in per contraction-chunk so tile-0 matmuls can begin
            # as soon as chunk 0 lands (after W_s, which gates the rank-1)
            for k in range(KC):
                nc.gpsimd.dma_start(out=whb[:, k, :], in_=wh_v[:, k, :])

            # coverage normalization
            cov_f = pro.tile([bpc, s], fp32, tag="covf")
            nc.sync.dma_start(out=cov_f, in_=cov_in[:])
            cov_sum = pro.tile([bpc, 1], fp32, tag="covsum")
            nc.vector.tensor_reduce(out=cov_sum, in_=cov_f, axis=mybir.AxisListType.X, op=add)
            nc.vector.tensor_scalar_add(cov_sum, cov_sum, 0.001 * s)
            cov_rcp = pro.tile([bpc, 1], fp32, tag="covrcp")
            nc.vector.reciprocal(cov_rcp, cov_sum)
            nc.vector.tensor_scalar(
                out=cov_nb, in0=cov_f, scalar1=0.001, scalar2=cov_rcp, op0=add, op1=mult
            )

        # ---- main loop --------------------------------------------------
        pending_epilogue = [None]
        for b in range(bpc):
            # rank-1 bias operands for this batch
            r1b = bpool.tile([2, D], bf16, tag="r1b")
            nc.sync.dma_start(out=r1b[0:1, :], in_=wstate_b[b : b + 1, :])
            nc.sync.dma_start(out=r1b[1:2, :], in_=wcov_bf)
            lcovb = bpool.tile([2, s], bf16, tag="lcovb")
            nc.vector.memset(lcovb[0:1, :], 1.0)
            nc.sync.dma_start(out=lcovb[1:2, :], in_=cov_nb[b : b + 1, :])
            # mask -> (P, nt) bf16 for this batch
            mi = bpool.tile([nt, P], i32, tag="maski")
            nc.sync.dma_start(out=mi, in_=mask_v[b])
            mf = bpool.tile([nt, P], fp32, tag="maskf")
            nc.vector.tensor_copy(mf, mi)
            mb = bpool.tile([nt, P], bf16, tag="maskb")
            nc.vector.tensor_copy(mb, mf)
            pmt = ps_tr.tile([P, nt], bf16, tag="tr")
            nc.tensor.transpose(pmt, mb, ident[:nt, :nt])
            mask_t = bpool.tile([P, nt], bf16, tag="maskt")
            nc.vector.tensor_copy(mask_t, pmt)

            # Per-tile: load context bf16, transpose 128x128 blocks on the PE
            # (software-pipelined one tile ahead so the PSUM->SBUF copies never
            # stall the PE), then GEMMs + fused softmax-numerator + attn_h
            # accumulation. No max-subtraction is needed (|e| is small and the
            # shift cancels), so p = exp(e)*mask accumulates into attn_h
            # immediately and the context tile is released after ~2 tiles.
            p_buf = bpool.tile([P, nt], bf16, tag="pbuf")
            pa = ps_attn.tile([1, D], fp32, tag="attn")
            nat_tiles = [None] * nt
            ctxT_tiles = [None] * nt

            def emit_T(t):
                if b == 0 and t < len(preload):
                    cn = preload[t]
                else:
                    cn = ctxp.tile([P, D], bf16, tag="nat")
                    nc.gpsimd.dma_start(out=cn, in_=ctx_in[b, t * P : (t + 1) * P, :])
                nat_tiles[t] = cn
                ptr = ps_tr.tile([P, KC, P], bf16, tag="tr")
                ctxTt = trp.tile([P, KC, P], bf16, tag="ctxT")
                ctxT_tiles[t] = ctxTt
                for k in range(KC):
                    nc.tensor.transpose(ptr[:, k, :], cn[:, k * P : (k + 1) * P], ident)
                    if k % 2 == 0:
                        nc.vector.tensor_copy(ctxTt[:, k, :], ptr[:, k, :])
                    else:
                        nc.scalar.copy(ctxTt[:, k, :], ptr[:, k, :])

            def emit_M(t):
                pm = ps_main.tile([P, D], fp32, tag="main")
                for n in range(D // NH):
                    nc.tensor.matmul(
                        pm[:, n * NH : (n + 1) * NH],
                        lhsT=lcovb[:, t * P : (t + 1) * P],
                        rhs=r1b[:, n * NH : (n + 1) * NH],
                        start=True,
                        stop=False,
                    )
                for k in range(KC):
                    for n in range(D // NH):
                        nc.tensor.matmul(
                            pm[:, n * NH : (n + 1) * NH],
                            lhsT=ctxT_tiles[t][:, k, :],
                            rhs=whb[:, k, n * NH : (n + 1) * NH],
                            start=False,
                            stop=(k == KC - 1),
                        )
                act = actp.tile([P, D], bf16, tag="act")
                nc.scalar.activation(act, pm, Tanh)
                ttro = scr.tile([P, D], bf16, tag="ttro")
                nc.vector.tensor_mul(ttro, act, vw_b)
                e_col = scr.tile([P, 1], fp32, tag="ecol")
                nc.vector.tensor_reduce(
                    out=e_col, in_=ttro, axis=mybir.AxisListType.X, op=add
                )
                nc.scalar.activation(p_buf[:, t : t + 1], e_col, Exp)
                nc.vector.tensor_mul(
                    p_buf[:, t : t + 1], p_buf[:, t : t + 1], mask_t[:, t : t + 1]
                )
                for tp in ([t - 1] if t >= 1 else []) + ([t] if t == nt - 1 else []):
                    for n in range(D // NH):
                        nc.tensor.matmul(
                            pa[:, n * NH : (n + 1) * NH],
                            lhsT=p_buf[:, tp : tp + 1],
                            rhs=nat_tiles[tp][:, n * NH : (n + 1) * NH],
                            start=(tp == 0),
                            stop=(tp == nt - 1),
                        )

            def make_epilogue(b=b, p_buf=p_buf, pa=pa):
                def epilogue():
                    # Z = sum over all positions (partition-sum via ones matmul)
                    zp = ps_tr.tile([1, nt], fp32, tag="tr")
                    nc.tensor.matmul(zp, lhsT=ones_col, rhs=p_buf, start=True, stop=True)
                    zs = outp.tile([1, 1], fp32, tag="zs")
                    nc.vector.tensor_reduce(
                        out=zs, in_=zp, axis=mybir.AxisListType.X, op=add
                    )
                    # broadcast Z to all partitions via a K=1 ones matmul, then 1/Z
                    zbp = ps_tr.tile([P, 1], fp32, tag="tr")
                    nc.tensor.matmul(zbp, lhsT=ones_row, rhs=zs, start=True, stop=True)
                    rzb = outp.tile([P, 1], fp32, tag="rzb")
                    nc.vector.reciprocal(rzb, zbp)
                    ah = outp.tile([1, D], fp32, tag="ah")
                    nc.vector.tensor_scalar_mul(ah, pa, rzb[0:1, :])
                    nc.sync.dma_start(out=ah_out[b : b + 1, :], in_=ah)
                    # attn row: scale (bf16), transpose to (t, p) for a
                    # contiguous DRAM write, cast fp32 on the PSUM->SBUF copy
                    arow = outp.tile([P, nt], bf16, tag="arow")
                    nc.vector.tensor_scalar_mul(arow, p_buf, rzb)
                    pat = ps_tr.tile([nt, P], bf16, tag="tr")
                    nc.tensor.transpose(pat, arow, ident)
                    arow_t = outp.tile([nt, P], fp32, tag="arowt")
                    nc.vector.tensor_copy(arow_t, pat)
                    nc.sync.dma_start(out=attn_v[b], in_=arow_t)
                return epilogue

            emit_T(0)
            for t in range(nt):
                if t + 1 < nt:
                    emit_T(t + 1)
                emit_M(t)
                if t == 0 and pending_epilogue[0] is not None:
                    pending_epilogue[0]()
                    pending_epilogue[0] = None
            pending_epilogue[0] = make_epilogue()

        pending_epilogue[0]()

    nc.compile()
    return nc


_NC_CACHE = {}


def _get_nc():
    if "nc" not in _NC_CACHE:
        _NC_CACHE["nc"] = build_nc()
    return _NC_CACHE["nc"]


def make_in_maps(inputs, bpc=BPC, ncores=NCORES):
    f = lambda x: np.ascontiguousarray(np.asarray(x, dtype=np.float32))
    shared = {
        "W_h": f(inputs["W_h"]),
        "b_h": f(inputs["b_h"]).reshape(1, D),
        "W_s": f(inputs["W_s"]),
        "b_s": f(inputs["b_s"]).reshape(1, D),
        "W_cov": f(inputs["W_cov"]).reshape(1, D),
        "b_cov": f(inputs["b_cov"]).reshape(1, D),
        "v_w": f(inputs["v_w"]).reshape(1, D),
    }
    mask = np.ascontiguousarray(np.asarray(inputs["context_mask"], dtype=np.int32))
    in_maps = []
    for c in range(ncores):
        sl = slice(c * bpc, (c + 1) * bpc)
        m = dict(shared)
        m["context"] = f(inputs["context"][sl])
        m["state_h"] = f(inputs["state_h"][sl])
        m["state_c"] = f(inputs["state_c"][sl])
        m["coverage"] = f(inputs["coverage"][sl])
        m["context_mask"] = mask[sl]
        in_maps.append(m)
    return in_maps


def gather_outputs(results):
    attn_h = np.concatenate([np.asarray(r["attn_h"]) for r in results], axis=0)
    attn = np.concatenate([np.asarray(r["attn"]) for r in results], axis=0)
    return attn_h.astype(np.float32), attn.astype(np.float32)


def kernel(**inputs):
    from concourse.bass_utils import run_bass_kernel_spmd

    nc = _get_nc()
    in_maps = make_in_maps(inputs)
    res = run_bass_kernel_spmd(nc, in_maps, core_ids=list(range(NCORES)))
    return gather_outputs(res.results)
